# revision 11
# baseline (speedup 1.0000x reference)
"""OS-CFAR 2D rank filter on 8 Trainium2 NeuronCores.

Per output pixel: take the 144 "training" cells of a 13x13 window with a
5x5 guard hole (circular padding), find the 36th largest (== value returned
by top_k(...,36)[...,-1]), multiply by ALPHA.

Strategy: spatially shard [512,1024] into 8 tiles of [128,512]
(4 row-bands x 2 col-halves) with 6-wide circular halos. Each core (raw
Bass, manual semaphores — all compute on the vector engine):
 - DMA a partition-shifted replicated slab rep[p, dy6*524 + x] =
   slab[p+dy6, x] into SBUF (engine APs must start at partition 0, so the
   dy shift is realized by the DMA's overlapping DRAM source rows)
 - pre-scale by ALPHA (monotone in f32 => order statistic commutes)
 - materialize per-pixel windows [128 pixels, 144 cells] in SBUF using
   sliding-window (overlapping) source access patterns, contiguous dx
   runs per dy
 - 5 rounds of vector.max (top-8) + vector.match_replace(-BIG) to extract
   ranks 1..40; the 36th largest is round-5 output index 3
 - DMA the assembled [128,512] answer out.
"""

import math

import numpy as np

# ---------------------------------------------------------------- constants
G = (2, 2)
T = (4, 4)
PFA = 1e-05
K = 108
N = 144          # ring training cells in 13x13 minus 5x5 guard
PW = 6           # halo width (G+T)
V, R = 512, 1024
SLAB_H, SLAB_W = 140, 524      # 128 + 2*PW, 512 + 2*PW
REP_W = 13 * SLAB_W            # replicated slab row length


def _log_factorial(n):
    n = n + 1
    if n < 9:
        return np.log(float(math.factorial(n)))
    return 0.5 * (np.log(2 * np.pi) - np.log(n)) + n * (
        np.log(n + 1.0 / (12.0 * n - 1.0 / (10.0 * n))) - 1.0
    )


def _fun(k, n, t, pfa):
    return (
        _log_factorial(n)
        - _log_factorial(n - k)
        - np.sum(np.log(np.arange(n, n - k, -1) + t))
        - np.log(pfa)
    )


def _os_cfar_threshold(k, n, pfa):
    lo, hi = 1.0, 1e32
    for _ in range(300):
        mid = 0.5 * (lo + hi)
        if _fun(k, n, mid, pfa) > 0:
            lo = mid
        else:
            hi = mid
    return 0.5 * (lo + hi)


ALPHA = float(np.float32(_os_cfar_threshold(K, N, PFA)))

NEG = -1e30
RANK = 36          # need the 36th largest of the 144 ring cells
ROUNDS = (RANK + 7) // 8                  # 5
LAST_IDX = RANK - 8 * (ROUNDS - 1) - 1    # 3

_CACHE = {}


def _runs_for(dy6):
    # contiguous dx6 runs of ring cells for window row dy6 (guard hole is
    # dy6 in 4..8 x dx6 in 4..8)
    if 4 <= dy6 <= 8:
        return [(0, 4), (9, 4)]
    return [(0, 13)]


def _build():
    import concourse.bass as bass
    import concourse.mybir as mybir
    from concourse.ap import AP

    f32 = mybir.dt.float32
    nc = bass.Bass(trn_type="TRN2")
    slab = nc.dram_tensor("slab", [SLAB_H, SLAB_W], f32, kind="ExternalInput")
    out = nc.dram_tensor("out", [128, 512], f32, kind="ExternalOutput")

    with (
        nc.sbuf_tensor([128, REP_W], f32) as rep,
        nc.sbuf_tensor([128, 128 * N], f32) as win,
        nc.sbuf_tensor([128, 16], f32) as scratch8,
        nc.sbuf_tensor([128, 128 * 8], f32) as final8,
        nc.sbuf_tensor([128, 512], f32) as ans,
        nc.semaphore() as dma_sem,
        nc.semaphore() as dve_sem,
        nc.Block() as block,
    ):

        @block.sync
        def _(sync):
            # load rep[p, dy6*524 + x] = slab[p + dy6, x], dy6 groups of 4
            for g0 in range(0, 13, 4):
                gc = min(4, 13 - g0)
                src = AP(
                    tensor=slab,
                    offset=g0 * SLAB_W,
                    ap=[[SLAB_W, 128], [SLAB_W, gc], [1, SLAB_W]],
                )
                dst = AP(
                    tensor=rep,
                    offset=g0 * SLAB_W,
                    ap=[[REP_W, 128], [SLAB_W, gc], [1, SLAB_W]],
                )
                sync.dma_start(dst, src).then_inc(dma_sem, 16)
            sync.wait_ge(dve_sem, 1)
            sync.dma_start(out[:, :], ans[:, :]).then_inc(dma_sem, 16)

        @block.vector
        def _(vector):
            vector.wait_ge(dma_sem, 16 * 4)
            # pre-scale by ALPHA (f32-monotone, so commutes with the order
            # statistic and matches reference rounding exactly)
            nc.vector.tensor_scalar_mul(rep[:, :], rep[:, :], ALPHA)

            for cx in range(4):
                # materialize windows: win[p, x*144 + o] = window cell o of
                # pixel (p, cx*128 + x)
                o = 0
                for dy6 in range(13):
                    for dx0, rl in _runs_for(dy6):
                        src = AP(
                            tensor=rep,
                            offset=dy6 * SLAB_W + cx * 128 + dx0,
                            ap=[[REP_W, 128], [1, 128], [1, rl]],
                        )
                        dst = AP(
                            tensor=win,
                            offset=o,
                            ap=[[128 * N, 128], [N, 128], [1, rl]],
                        )
                        nc.vector.tensor_copy(dst, src)
                        o += rl
                assert o == N

                # 2-way interleave so a match_replace never directly follows
                # the max that wrote its needles (same-engine SBUF write
                # latency makes the back-to-back pair read stale needles)
                for xi in range(0, 128, 2):
                    wj = [win[:, (xi + j) * N : (xi + j + 1) * N] for j in range(2)]
                    sj = [scratch8[:, 8 * j : 8 * j + 8] for j in range(2)]
                    fj = [final8[:, (xi + j) * 8 : (xi + j + 1) * 8] for j in range(2)]
                    for rnd in range(ROUNDS):
                        for j in range(2):
                            nc.vector.max(
                                out=(fj[j] if rnd == ROUNDS - 1 else sj[j]),
                                in_=wj[j],
                            )
                        if rnd < ROUNDS - 1:
                            for j in range(2):
                                nc.vector.match_replace(
                                    out=wj[j],
                                    in_to_replace=sj[j],
                                    in_values=wj[j],
                                    imm_value=NEG,
                                )

                # gather the 36th largest of each pixel into ans
                src = AP(
                    tensor=final8,
                    offset=LAST_IDX,
                    ap=[[128 * 8, 128], [8, 128]],
                )
                ins = nc.vector.tensor_copy(ans[:, cx * 128 : (cx + 1) * 128], src)
                if cx == 3:
                    ins.then_inc(dve_sem, 1)

    return nc


def kernel(data: np.ndarray) -> np.ndarray:
    from concourse.bass_utils import run_bass_kernel_spmd

    img = np.asarray(data, dtype=np.float32)[0]          # [512,1024]
    pad = np.pad(img, PW, mode="wrap")                    # [524,1036]

    if "nc" not in _CACHE:
        _CACHE["nc"] = _build()
    nc = _CACHE["nc"]

    in_maps = []
    for c in range(8):
        band, half = c // 2, c % 2
        rb, cb = band * 128, half * 512
        in_maps.append(
            {"slab": np.ascontiguousarray(pad[rb : rb + SLAB_H, cb : cb + SLAB_W])}
        )

    res = run_bass_kernel_spmd(nc, in_maps, core_ids=list(range(8)))

    full = np.empty((V, R), dtype=np.float32)
    for c in range(8):
        band, half = c // 2, c % 2
        full[band * 128 : (band + 1) * 128, half * 512 : (half + 1) * 512] = (
            res.results[c]["out"]
        )
    return full
